# revision 1
# baseline (speedup 1.0000x reference)
"""MoE block (grouped GEMM x2 + SwiGLU) for 8 Trainium2 NeuronCores.

Expert-parallel: 8 experts per core, tokens routed on host (inputs are
pre-sorted by expert), no on-device collectives. Per core, for each of its
8 experts e and each I-chunk i (128 wide):
  GEMM1 (PE):  psum_gu[tok=128, 256] += xT[d,tok].T @ w13[d, (gate_i|up_i)]
               accumulated over 16 d-chunks of 128
  SwiGLU:      silu(gate) (ACT) * up (DVE) -> h[tok=128, 128]
  transpose:   h -> hT[128, tok] (PE, via identity)
  GEMM2 (PE):  psum_y[tok=128, 2048] += hT.T @ w2[i-chunk, :]
               accumulated over the 11 I-chunks
Weights stream through SBUF in ~2MB/1MB contiguous DMAs (the kernel is
memory-bound: ~293MB of weights+acts per core).
"""

import sys

sys.path.insert(0, "/opt/trn_rl_repo")

import numpy as np

import concourse.bass as bass
import concourse.mybir as mybir
import concourse.tile as tile
from concourse import bacc
from concourse.bass_utils import run_bass_kernel_spmd
from concourse.masks import make_identity

E = 64
D = 2048
I = 1408
T = 8192
NCORES = 8
EPC = E // NCORES  # experts per core
P = 128

F32 = mybir.dt.float32

_prog_cache = {}


def build_nc(C=128, d=D, i_dim=I, epc=EPC, mode="f32"):
    """Build the single-core SPMD program.

    C: token capacity per expert (multiple of 128).
    mode: "f32" (exact, PE-bound) | "f32r" (TF32-like matmul, rel-err ~2e-4)
        | "bf16" (bf16-staged weights/x, rel-err ~4e-3, half the DMA bytes)
    """
    nd = d // P           # contraction chunks for GEMM1
    ni = i_dim // P       # I chunks
    tt = C // P           # token tiles per expert
    g2n = 512 if d % 512 == 0 else P  # GEMM2 output column chunk width
    ndd = d // g2n
    assert d % P == 0 and i_dim % P == 0 and C % P == 0

    mm_dt = {"f32": F32, "f32r": mybir.dt.float32r,
             "bf16": mybir.dt.bfloat16}[mode]

    nc = bacc.Bacc(None, target_bir_lowering=False)
    xt = nc.dram_tensor("xt", [epc, P, nd, C], mm_dt, kind="ExternalInput")
    w13 = nc.dram_tensor("w13", [epc, ni, P, nd, 256], mm_dt, kind="ExternalInput")
    w2 = nc.dram_tensor("w2", [epc, ni, P, d], mm_dt, kind="ExternalInput")
    y_dt = F32
    y = nc.dram_tensor("y", [epc * C, d], y_dt, kind="ExternalOutput")

    with tile.TileContext(nc) as tc:
        with (
            tc.tile_pool(name="singles", bufs=1) as singles,
            tc.tile_pool(name="xpool", bufs=3) as xpool,
            tc.tile_pool(name="w13pool", bufs=4) as w13pool,
            tc.tile_pool(name="w2pool", bufs=3) as w2pool,
            tc.tile_pool(name="hpool", bufs=3) as hpool,
            tc.tile_pool(name="ypool", bufs=2) as ypool,
            tc.tile_pool(name="psgu", bufs=2, space="PSUM") as psgu,
            tc.tile_pool(name="pst", bufs=2, space="PSUM") as pst,
            tc.tile_pool(name="psy", bufs=1, space="PSUM") as psy,
        ):
            ident_f32 = singles.tile([P, P], F32)
            make_identity(nc, ident_f32)
            if mode == "f32":
                ident = ident_f32
            else:
                ident = singles.tile([P, P], mm_dt)
                nc.vector.tensor_copy(ident, ident_f32)

            WG = 2   # I-chunks per w13 DMA (>=1MB transfers at bf16)
            WG2 = 4  # I-chunks per w2 DMA
            for e in range(epc):
                xe = xpool.tile([P, nd, C], mm_dt, tag="xe")
                nc.sync.dma_start(out=xe, in_=xt[e])
                for t in range(tt):
                    pye = psy.tile([P, d], F32, tag="py")
                    for i0 in range(0, ni, WG):
                        gsz = min(WG, ni - i0)
                        wt = w13pool.tile([P, WG, nd, 256], mm_dt, tag="w13t")
                        nc.sync.dma_start(
                            out=wt[:, :gsz],
                            in_=w13[e, i0:i0 + gsz].rearrange("i p k c -> p i k c"),
                        )
                        if i0 % WG2 == 0:
                            g2sz = min(WG2, ni - i0)
                            w2t = w2pool.tile([P, WG2, d], mm_dt, tag="w2t")
                            nc.sync.dma_start(
                                out=w2t[:, :g2sz],
                                in_=w2[e, i0:i0 + g2sz].rearrange("i p f -> p i f"),
                            )
                        for j in range(gsz):
                            i = i0 + j
                            pgu = psgu.tile([P, 256], F32, tag="pgu")
                            for k in range(nd):
                                nc.tensor.matmul(
                                    pgu,
                                    lhsT=xe[:, k, t * P:(t + 1) * P],
                                    rhs=wt[:, j, k, :],
                                    start=(k == 0),
                                    stop=(k == nd - 1),
                                )
                            sg = hpool.tile([P, P], F32, tag="sg")
                            nc.scalar.activation(
                                sg, pgu[:, 0:P],
                                mybir.ActivationFunctionType.Sigmoid,
                            )
                            h1 = hpool.tile([P, P], F32, tag="h1")
                            nc.vector.tensor_mul(h1, sg, pgu[:, P:256])
                            h = hpool.tile([P, P], mm_dt, tag="h")
                            nc.vector.tensor_mul(h, h1, pgu[:, 0:P])
                            pt = pst.tile([P, P], mm_dt, tag="pt")
                            nc.tensor.transpose(pt, h, ident)
                            hT = hpool.tile([P, P], mm_dt, tag="hT")
                            nc.vector.tensor_copy(hT, pt)
                            for dd in range(ndd):
                                nc.tensor.matmul(
                                    pye[:, dd * g2n:(dd + 1) * g2n],
                                    lhsT=hT,
                                    rhs=w2t[:, i % WG2, dd * g2n:(dd + 1) * g2n],
                                    start=(i == 0),
                                    stop=(i == ni - 1),
                                )
                    ysb = ypool.tile([P, d], y_dt, tag="ysb")
                    nc.vector.tensor_copy(ysb, pye)
                    # y goes out on the gpsimd (SWDGE) queue: it depends on
                    # this expert's full compute drain, and on the sync queue
                    # it would head-of-line-block the next expert's weight
                    # DMAs at every expert boundary.
                    nc.gpsimd.dma_start(
                        out=y[e * C + t * P:e * C + (t + 1) * P, :], in_=ysb
                    )
    nc.compile()
    return nc


def _host_shard(x, counts, w13, w2, C, np_dt=np.float32):
    """Build per-core input maps (arrays staged in np_dt)."""
    offs = np.zeros(E + 1, np.int64)
    np.cumsum(counts, out=offs[1:])
    in_maps = []
    for c in range(NCORES):
        xt_c = np.zeros((EPC, P, D // P, C), np_dt)
        for le in range(EPC):
            g = c * EPC + le
            cnt = int(counts[g])
            if cnt:
                xe = x[offs[g]:offs[g] + cnt]            # [cnt, D]
                xe = xe.reshape(cnt, D // P, P)           # t, do, di
                xt_c[le, :, :, :cnt] = xe.transpose(2, 1, 0).astype(np_dt)
        wsl = w13[c * EPC:(c + 1) * EPC]                  # [EPC, D, 2I]
        # [EPC, do, di, g, i, f] -> [EPC, i, di, do, (g f)]
        w13_c = np.ascontiguousarray(
            wsl.reshape(EPC, D // P, P, 2, I // P, P)
            .transpose(0, 4, 2, 1, 3, 5)
            .reshape(EPC, I // P, P, D // P, 256)
            .astype(np_dt, copy=False)
        )
        w2_c = np.ascontiguousarray(
            w2[c * EPC:(c + 1) * EPC].reshape(EPC, I // P, P, D)
            .astype(np_dt, copy=False)
        )
        in_maps.append({"xt": xt_c, "w13": w13_c, "w2": w2_c})
    return in_maps, offs


def kernel(x, tokens_per_expert, decoding, w13, w2, _trace=False, _mode="bf16"):
    x = np.asarray(x, dtype=np.float32)
    counts = np.asarray(tokens_per_expert, dtype=np.int64)
    w13 = np.asarray(w13, dtype=np.float32)
    w2 = np.asarray(w2, dtype=np.float32)

    C = max(P, int(-(-max(counts.max(), 1) // P)) * P)

    key = (C, _mode)
    if key not in _prog_cache:
        _prog_cache[key] = build_nc(C=C, mode=key[1])
    nc = _prog_cache[key]

    if _mode == "bf16":
        import ml_dtypes
        np_dt = ml_dtypes.bfloat16
    else:
        np_dt = np.float32
    in_maps, offs = _host_shard(x, counts, w13, w2, C, np_dt=np_dt)
    res = run_bass_kernel_spmd(
        nc, in_maps, list(range(NCORES)), trace=_trace
    )

    out = np.zeros((int(counts.sum()), D), np.float32)
    for c in range(NCORES):
        yc = np.asarray(res.results[c]["y"], dtype=np.float32)
        for le in range(EPC):
            g = c * EPC + le
            cnt = int(counts[g])
            if cnt:
                out[offs[g]:offs[g] + cnt] = yc[le * C:le * C + cnt]
    if _trace:
        return out, res
    return out



# revision 9
# speedup vs baseline: 1.5065x; 1.5065x over previous
"""MoE block (grouped GEMM x2 + SwiGLU) for 8 Trainium2 NeuronCores.

Expert-parallel: 8 experts per core, tokens routed on host (inputs are
pre-sorted by expert), no on-device collectives.

Memory-bound kernel: weight bytes dominate, so weights are staged in
reduced precision chosen to stay inside the rel-err budget (<2e-2):
  - w13 gate half: fp16 (sigmoid input needs accuracy; fp16 costs the
    same DMA bytes as bf16 but 8x less rounding error)
  - w13 up half:   fp8 e3m4 (x64 scale), except the last N16 i-chunks
    kept fp16 for error margin
  - w2:            fp8 e3m4 (x64 scale), except the last N16 i-chunks
  - x, h, y:       fp16
Per-tensor quantization error (e3m4 ~1.3% rms) combines to ~1.8e-2
total rel err (validated against the reference on CPU).

Per core, for each of its 8 experts e, i-chunk pairs (i0,i0+1):
  GEMM1 (PE):  pgu[tok=128, 0:256]   += xT[d,tok].T @ wg[d, pair]  (fp16)
               pgu[tok=128, 256:512] += xT[d,tok].T @ wu[d, pair]  (e3m4)
               over 16 d-chunks of 128; 256-wide movings amortize the
               ~40ns/instr PE overhead
  SwiGLU:      sg=sigmoid(g) (ACT); h = sg*u_hat*g (DVE) -> fp16, =64*h
  transpose:   h -> hT[128, tok] (PE, via identity)
  GEMM2 (PE):  psum_y[tok, 2048] += hT.T @ w2[i, :]  (fp16 x e3m4)
               accumulated over the 11 I-chunks; psum carries 4096*y,
               rescaled 1/4096 on the psum->sbuf copy.
Weights stream through SBUF in 0.5-2.9MB DMAs with >=4KB contiguous
runs per partition (sync HWDGE queue); y leaves on the gpsimd queue.
"""

import sys

sys.path.insert(0, "/opt/trn_rl_repo")

import numpy as np

import concourse.bass as bass
import concourse.mybir as mybir
import concourse.tile as tile
from concourse import bacc
from concourse.bass_utils import run_bass_kernel_spmd
from concourse.masks import make_identity

E = 64
D = 2048
I = 1408
T = 8192
NCORES = 8
EPC = E // NCORES  # experts per core
P = 128
ND = D // P        # 16 contraction chunks for GEMM1
NI = I // P        # 11 I-chunks
WSCALE = 64.0      # power-of-2 scale on up/w2 weights (e3m4 range fit)

F32 = mybir.dt.float32
F16 = mybir.dt.float16
E3M4 = mybir.dt.float8e3

_prog_cache = {}


def build_mix(C=128, n16=1):
    """Single-core SPMD program. C: token capacity per expert (mult of 128).
    n16: number of trailing i-chunks of up/w2 kept in fp16 (error margin)."""
    tt = C // P
    ne3 = NI - n16     # leading i-chunks in e3m4
    assert C % P == 0 and 0 <= n16 <= NI
    HK = ND // 2       # k-chunks per weight DMA half

    nc = bacc.Bacc(None, target_bir_lowering=False)
    xt = nc.dram_tensor("xt", [EPC, P, ND, C], F16, kind="ExternalInput")
    # i-chunk-major layouts: one [P, all-k] slab per i-chunk so weights can
    # stream at i-pair granularity and compute trails DMA by only one pair
    wg = nc.dram_tensor("wg", [EPC, NI, P, ND * P], F16, kind="ExternalInput")
    wu = (nc.dram_tensor("wu", [EPC, ne3, P, ND * P], E3M4, kind="ExternalInput")
          if ne3 else None)
    wu16 = (nc.dram_tensor("wu16", [EPC, n16, P, ND * P], F16, kind="ExternalInput")
            if n16 else None)
    w2e = (nc.dram_tensor("w2e", [EPC, P, ne3, D], E3M4, kind="ExternalInput")
           if ne3 else None)
    w2h = (nc.dram_tensor("w2h", [EPC, P, n16, D], F16, kind="ExternalInput")
           if n16 else None)
    y = nc.dram_tensor("y", [EPC * C, D], F16, kind="ExternalOutput")

    # i-chunk groups: pairs within the e3m4 range, then fp16 singles
    groups = [(i, min(2, ne3 - i)) for i in range(0, ne3, 2)]
    groups += [(i, 1) for i in range(ne3, NI)]

    with tile.TileContext(nc) as tc:
        with (
            tc.tile_pool(name="singles", bufs=1) as singles,
            tc.tile_pool(name="xpool", bufs=2) as xpool,
            tc.tile_pool(name="wgpool", bufs=3) as wgpool,
            tc.tile_pool(name="wupool", bufs=3) as wupool,
            tc.tile_pool(name="wu16pool", bufs=2) as wu16pool,
            tc.tile_pool(name="w2pool", bufs=3) as w2pool,
            tc.tile_pool(name="w216pool", bufs=2) as w216pool,
            tc.tile_pool(name="hpool", bufs=3) as hpool,
            tc.tile_pool(name="ypool", bufs=2) as ypool,
            tc.tile_pool(name="psgu", bufs=2, space="PSUM") as psgu,
            tc.tile_pool(name="pst", bufs=2, space="PSUM") as pst,
            tc.tile_pool(name="psy", bufs=1, space="PSUM") as psy,
        ):
            ident_f32 = singles.tile([P, P], F32)
            make_identity(nc, ident_f32)
            ident = singles.tile([P, P], F16)
            nc.vector.tensor_copy(ident, ident_f32)

            for e in range(EPC):
                xe = xpool.tile([P, ND, C], F16, tag="xe")
                nc.sync.dma_start(out=xe, in_=xt[e])
                wgt = {}
                wut = {}
                w2t = {}
                for (i0, gw) in groups:
                    wgp = wgpool.tile([P, gw, ND * P], F16, tag="wgt", bufs=6,
                                      name="wgt", padded_shape=[P, 2, ND * P])
                    nc.sync.dma_start(
                        out=wgp,
                        in_=wg[e, i0:i0 + gw].rearrange("i p c -> p i c"))
                    wgt[i0] = wgp
                    if i0 < ne3:
                        wup = wupool.tile([P, gw, ND * P], E3M4, tag="wut",
                                          bufs=6, name="wut",
                                          padded_shape=[P, 2, ND * P])
                        nc.sync.dma_start(
                            out=wup,
                            in_=wu[e, i0:i0 + gw].rearrange("i p c -> p i c"))
                        wut[i0] = wup
                        w2p = w2pool.tile([P, gw, D], E3M4, tag="w2t", bufs=6,
                                          name="w2t", padded_shape=[P, 2, D])
                        nc.sync.dma_start(out=w2p, in_=w2e[e][:, i0:i0 + gw])
                        w2t[i0] = w2p
                    elif i0 == ne3:
                        wu16t = wu16pool.tile([P, n16, ND * P], F16, tag="wu16")
                        nc.sync.dma_start(
                            out=wu16t,
                            in_=wu16[e].rearrange("i p c -> p i c"))
                        w216t = w216pool.tile([P, n16, D], F16, tag="w216")
                        nc.sync.dma_start(out=w216t, in_=w2h[e])

                for t in range(tt):
                    ts = slice(t * P, (t + 1) * P)
                    pye = psy.tile([P, D], F32, tag="py")

                    def gemm1(i0, gw):
                        pgu = psgu.tile([P, 4 * P], F32, tag="pgu")
                        for k in range(ND):
                            nc.tensor.matmul(
                                pgu[:, 0:gw * P],
                                lhsT=xe[:, k, ts],
                                rhs=wgt[i0][:, :, k * P:(k + 1) * P],
                                start=(k == 0), stop=(k == ND - 1),
                            )
                        for k in range(ND):
                            if i0 < ne3:
                                urhs = wut[i0][:, :, k * P:(k + 1) * P]
                            else:
                                urhs = wu16t[:, i0 - ne3:i0 - ne3 + gw,
                                             k * P:(k + 1) * P]
                            nc.tensor.matmul(
                                pgu[:, 2 * P:(2 + gw) * P],
                                lhsT=xe[:, k, ts],
                                rhs=urhs,
                                start=(k == 0), stop=(k == ND - 1),
                            )
                        return pgu

                    def swiglu(pgu, gw):
                        sg = hpool.tile([P, gw * P], F32, tag="sg")
                        nc.scalar.activation(
                            sg, pgu[:, 0:gw * P],
                            mybir.ActivationFunctionType.Sigmoid,
                        )
                        h1 = hpool.tile([P, gw * P], F32, tag="h1")
                        nc.vector.tensor_mul(h1, sg, pgu[:, 2 * P:(2 + gw) * P])
                        h = hpool.tile([P, gw * P], F16, tag="h")
                        nc.vector.tensor_mul(h, h1, pgu[:, 0:gw * P])
                        return h

                    def gemm2(i0, gw, h):
                        for j in range(gw):
                            i = i0 + j
                            pt = pst.tile([P, P], F16, tag="pt")
                            nc.tensor.transpose(pt, h[:, j * P:(j + 1) * P], ident)
                            hT = hpool.tile([P, P], F16, tag="hT")
                            nc.vector.tensor_copy(hT, pt)
                            if i < ne3:
                                w2slab = w2t[i0][:, j]
                            else:
                                w2slab = w216t[:, i - ne3]
                            for dd in range(D // 512):
                                nc.tensor.matmul(
                                    pye[:, dd * 512:(dd + 1) * 512],
                                    lhsT=hT,
                                    rhs=w2slab[:, dd * 512:(dd + 1) * 512],
                                    start=(i == 0), stop=(i == NI - 1),
                                )

                    prev = None
                    for (i0, gw) in groups:
                        pgu = gemm1(i0, gw)
                        if prev is not None:
                            gemm2(prev[0], prev[1], prev[2])
                        h = swiglu(pgu, gw)
                        prev = (i0, gw, h)
                    gemm2(prev[0], prev[1], prev[2])

                    ysb = ypool.tile([P, D], F16, tag="ysb")
                    nc.vector.tensor_scalar_mul(ysb, pye, 1.0 / (WSCALE * WSCALE))
                    # y depends on the full compute drain; keep it off the
                    # sync queue so it can't head-of-line-block weight DMAs
                    nc.gpsimd.dma_start(
                        out=y[e * C + t * P:e * C + (t + 1) * P, :], in_=ysb
                    )
    nc.compile()
    return nc


def _host_shard(x, counts, w13, w2, C, n16):
    """Build per-core input maps for the mixed-precision layout."""
    import ml_dtypes
    e3 = ml_dtypes.float8_e3m4
    ne3 = NI - n16
    offs = np.zeros(E + 1, np.int64)
    np.cumsum(counts, out=offs[1:])
    in_maps = []
    for c in range(NCORES):
        xt_c = np.zeros((EPC, P, ND, C), np.float16)
        for le in range(EPC):
            g = c * EPC + le
            cnt = int(counts[g])
            if cnt:
                xe = x[offs[g]:offs[g] + cnt]             # [cnt, D]
                xe = xe.reshape(cnt, ND, P)               # t, k, p
                xt_c[le, :, :, :cnt] = xe.transpose(2, 1, 0).astype(np.float16)
        wsl = w13[c * EPC:(c + 1) * EPC]                  # [EPC, D, 2I]
        gate = wsl[:, :, :I]                              # [EPC, D, I]
        up = wsl[:, :, I:]
        # [EPC, D(=k*P+p), nch*P] -> [EPC, nch, p, k*P] (i-chunk-major slabs)
        def imajor(a, nch, dt, scale=1.0):
            a = a.reshape(EPC, ND, P, nch, P).transpose(0, 3, 2, 1, 4)
            a = np.ascontiguousarray(a).reshape(EPC, nch, P, ND * P)
            if scale != 1.0:
                a = a * scale
            return a.astype(dt)
        m = {"xt": xt_c, "wg": imajor(gate, NI, np.float16)}
        if ne3:
            m["wu"] = imajor(up[:, :, :ne3 * P], ne3, e3, WSCALE)
        if n16:
            m["wu16"] = imajor(up[:, :, ne3 * P:], n16, np.float16, WSCALE)
        w2sl = w2[c * EPC:(c + 1) * EPC].reshape(EPC, NI, P, D)
        if ne3:
            m["w2e"] = (np.ascontiguousarray(
                w2sl[:, :ne3].transpose(0, 2, 1, 3)) * WSCALE).astype(e3)
        if n16:
            m["w2h"] = (np.ascontiguousarray(
                w2sl[:, ne3:].transpose(0, 2, 1, 3)) * WSCALE).astype(np.float16)
        in_maps.append(m)
    return in_maps, offs


def kernel(x, tokens_per_expert, decoding, w13, w2, _trace=False, _n16=1):
    x = np.asarray(x, dtype=np.float32)
    counts = np.asarray(tokens_per_expert, dtype=np.int64)
    w13 = np.asarray(w13, dtype=np.float32)
    w2 = np.asarray(w2, dtype=np.float32)

    C = max(P, int(-(-max(counts.max(), 1) // P)) * P)

    key = (C, _n16)
    if key not in _prog_cache:
        _prog_cache[key] = build_mix(C=C, n16=_n16)
    nc = _prog_cache[key]

    in_maps, offs = _host_shard(x, counts, w13, w2, C, _n16)
    res = run_bass_kernel_spmd(
        nc, in_maps, list(range(NCORES)), trace=_trace
    )

    out = np.zeros((int(counts.sum()), D), np.float32)
    for c in range(NCORES):
        yc = np.asarray(res.results[c]["y"], dtype=np.float32)
        for le in range(EPC):
            g = c * EPC + le
            cnt = int(counts[g])
            if cnt:
                out[offs[g]:offs[g] + cnt] = yc[le * C:le * C + cnt]
    if _trace:
        return out, res
    return out
